# revision 38
# baseline (speedup 1.0000x reference)
"""Trainium2 Bass kernel for nn_BinaryTokenClassificationModel (segment_reduce).

Math: the reference pools token embeddings into word embeddings (mean over
contiguous runs of equal word ids), then computes
    logits[b,s,t] = src_pooled[b,s] @ w_src + tgt_pooled[b,t] @ w_tgt + b.
Because the classifier is linear, pooling and projection commute:
    src_proj[w] = sum_t A[w,t] * (tok_h[t] @ w_src)     (A = 1/count-weighted
    tgt_proj[w] = sum_t A[w,t] * (tok_h[t] @ w_tgt)      segment membership)
and the output is the outer sum src_proj[s] + tgt_proj[t] + b.

Device pipeline (data-parallel, core i = batch row i, no collectives):
  - tok_h is shipped in bf16 (rel-err budget 2e-2; bf16 lands ~3e-3), halving
    the dominant HBM traffic. The two HWDGE queues stream in parallel, with
    chunk 0's dependencies leading different queues so it starts ~1us early:
    SP carries w_src + tok1..3, Act carries tok0 + w_tgt + membership + bias.
  - wb / atw are precomputed host-side: wb[p, :] = [w_src | w_tgt] for every
    partition p (replication only), atw[tok, word] = 1/count membership.
    This removes the on-device GpSimd broadcast + iota/compare chain that
    previously gated the whole pipeline.
  - per chunk: u_c[t] = tok_c[t, :] . w  via fused DVE multiply-reduce
    (f32 accumulate), then one bf16 TensorE matmul accumulates
    atw_c^T @ broadcast(u_c) (src) or broadcast(u_c) @ atw_c (tgt)
    into the [S, T] f32 PSUM tile = segment-reduce + outer-sum fused.
  - a dummy DVE custom op at program start pre-loads the DVE uop table so the
    first real mul-reduce doesn't pay the ~1us lazy table load.
  - output is DMA'd back in bf16 and upcast on host.
"""

import functools

import numpy as np
import ml_dtypes

import concourse.bacc as bacc
import concourse.mybir as mybir
from concourse.bass_utils import run_bass_kernel_spmd
from concourse.tile import TileContext

# Problem geometry (hardcoded per spec)
B = 8
L_SRC = 256
L_TGT = 256
L = L_SRC + L_TGT  # 512
H = 768
P = 128            # SBUF partitions / tokens per chunk
NCHUNK = L // P    # 4
N_SRC_CHUNKS = L_SRC // P  # 2
N_CORES = 8
F32 = mybir.dt.float32
BF16 = mybir.dt.bfloat16
NPBF16 = ml_dtypes.bfloat16


# ---------------------------------------------------------------------------
# Host-side segment bookkeeping (exact mirror of reference._pool_words)
# ---------------------------------------------------------------------------

def _segments(combined_wid, attention_mask, n_words):
    """Per-token dense run ids exactly as the reference computes them."""
    valid = (attention_mask > 0) & (combined_wid >= 0)  # [B, L]
    prev_wid = np.concatenate(
        [np.full((combined_wid.shape[0], 1), -2, dtype=combined_wid.dtype),
         combined_wid[:, :-1]], axis=1)
    prev_valid = np.concatenate(
        [np.zeros((valid.shape[0], 1), dtype=bool), valid[:, :-1]], axis=1)
    new_run = valid & ((combined_wid != prev_wid) | (~prev_valid))
    run_id = np.cumsum(new_run.astype(np.int64), axis=1) - 1  # [B, L]
    seg = np.where(valid, run_id, n_words)  # n_words = dummy slot
    return seg, valid


def _seg_weights(seg, valid, n_words):
    """1/max(count,1) weight for each token's segment (0 for invalid)."""
    Bv, Lv = seg.shape
    wgt = np.zeros((Bv, Lv), dtype=np.float32)
    for b in range(Bv):
        counts = np.bincount(seg[b][valid[b]], minlength=Lv + 1).astype(np.float32)
        inv = 1.0 / np.maximum(counts, 1.0)
        wgt[b] = np.where(valid[b] & (seg[b] < n_words), inv[np.minimum(seg[b], Lv)], 0.0)
    return wgt


# ---------------------------------------------------------------------------
# Device kernel, fast path (block_ok): src tokens -> word rows [0,S),
# tgt tokens -> word rows [S,S+T)
# ---------------------------------------------------------------------------

def _declare_block_params(nc, S, T, wb_mode):
    prm = dict(
        tok0=nc.declare_dram_parameter("tok0", [P, H], BF16, isOutput=False),
        tok1=nc.declare_dram_parameter("tok1", [P, H], BF16, isOutput=False),
        tok2=nc.declare_dram_parameter("tok2", [P, H], BF16, isOutput=False),
        tok3=nc.declare_dram_parameter("tok3", [P, H], BF16, isOutput=False),
        # atwb[p, c*P + w] = (seg[c*P+p] == block word w) * wgt[c*P+p]
        atwb=nc.declare_dram_parameter("atwb", [P, NCHUNK * P], BF16, isOutput=False),
        bias=nc.declare_dram_parameter("bias", [P, 1], F32, isOutput=False),
        out=nc.declare_dram_parameter("out", [S, T], BF16, isOutput=True),
    )
    if wb_mode == "dma":
        # wb_*[p, :] = w_* for every partition p (host-side broadcast); split
        # so w_src can land (and unblock chunk 0) before w_tgt streams.
        prm["wb_src"] = nc.declare_dram_parameter(
            "wb_src", [P, H], BF16, isOutput=False)
        prm["wb_tgt"] = nc.declare_dram_parameter(
            "wb_tgt", [P, H], BF16, isOutput=False)
    else:
        # tiny concatenated weight row, broadcast on-device by TensorE
        prm["wcat"] = nc.declare_dram_parameter(
            "wcat", [1, 2 * H], BF16, isOutput=False)
    return prm


def _emit_block_body(nc, tc, prm, S, T, mm_mode="mat", wb_mode="dma",
                     mr_op="amr"):
    tok_drams = [prm["tok0"], prm["tok1"], prm["tok2"], prm["tok3"]]
    atwb, bias, out = prm["atwb"], prm["bias"], prm["out"]
    with (
        tc.tile_pool(name="const", bufs=1) as cpool,
        tc.tile_pool(name="toks", bufs=4) as tpool,
        tc.tile_pool(name="prods", bufs=2) as ppool,
        tc.tile_pool(name="psum", bufs=1, space="PSUM") as pspool,
    ):
        if mr_op == "amr":
            # DVE uop-table warmup: tiny dummy custom op on a memset tile so
            # the first real affine_mul_reduce doesn't pay the lazy table load.
            warm = cpool.tile([1, 64], BF16)
            warm_acc = cpool.tile([1, 1], F32)
            nc.vector.memset(warm[:], 0.0)
            nc.vector.affine_mul_reduce(
                out=warm[:], accum_out=warm_acc[:], in0=warm[:], in1=warm[:],
                scale=1.0, bias=0.0)

        # --- Two HWDGE queues, balanced, in consumption order. Chunk 0's
        # dependencies (tok0 + w_src) lead DIFFERENT queues so both land
        # ~1us earlier than if they shared one stream.
        #   SP : wb_src | tok1 | tok2 | tok3
        #   Act: tok0 | wb_tgt | atwb | bias | (out)
        tok_sb = [tpool.tile([P, H], BF16, name=f"tok_{c}")
                  for c in range(NCHUNK)]
        atwb_sb = cpool.tile([P, NCHUNK * P], BF16)
        bias_sb = cpool.tile([P, 1], F32)
        with tc.high_priority():
            if wb_mode == "dma":
                # Per-queue order = consumption order of the DVE chain; the
                # late chunks (tok2/tok3) lead DIFFERENT queues so they land
                # concurrently, and atwb/bias (only needed by the matmul/
                # copy tail) ride behind them.
                #   Act : tok0 | tok2 | bias | (out)
                #   SP  : wb_src | tok1 | tok3
                #   Pool: wb_tgt | atwb  (SWDGE; both have relaxed deadlines
                #         -- c2's mul-reduce and the matmul tail -- and the
                #         Pool engine is idle anyway)
                # Note: the three queues SHARE the per-core HBM allocation
                # (~220B/ns with 8 cores streaming), so the choreography only
                # decides which dependency starves; total pre-tail bytes is
                # the real pacing term. SWDGE measured ~110-200B/ns -- fine
                # for the two deadline-relaxed transfers, too slow for a
                # token chunk.
                wbs_sb = cpool.tile([P, H], BF16)
                wbt_sb = cpool.tile([P, H], BF16)
                nc.scalar.dma_start(out=tok_sb[0][:], in_=tok_drams[0][:])
                nc.sync.dma_start(out=wbs_sb[:], in_=prm["wb_src"][:])
                nc.gpsimd.dma_start(out=wbt_sb[:], in_=prm["wb_tgt"][:])
                nc.sync.dma_start(out=tok_sb[1][:], in_=tok_drams[1][:])
                nc.scalar.dma_start(out=tok_sb[2][:], in_=tok_drams[2][:])
                nc.sync.dma_start(out=tok_sb[3][:], in_=tok_drams[3][:])
                nc.gpsimd.dma_start(out=atwb_sb[:], in_=atwb[:])
                nc.scalar.dma_start(out=bias_sb[:], in_=bias[:])
            else:
                # wb via TensorE broadcast: tiny wcat leads the SP queue
                # (starts streaming ~0.6us before Act); the ones-matmul
                # broadcast finishes while tok0 streams. 394KB less HBM
                # traffic than host-broadcast wb.
                #   SP : wcat | tok1 | tok2   (SP streams faster; traces show
                #   Act: tok0 | tok3 | bias    the Act queue's 2nd transfer
                #   Pool: atwb                 lands ~1.5us late, so the
                #                              later-deadline tok3 rides Act)
                wcat_sb = cpool.tile([1, 2 * H], BF16)
                nc.sync.dma_start(out=wcat_sb[:], in_=prm["wcat"][:])
                nc.scalar.dma_start(out=tok_sb[0][:], in_=tok_drams[0][:])
                nc.sync.dma_start(out=tok_sb[1][:], in_=tok_drams[1][:])
                nc.sync.dma_start(out=tok_sb[2][:], in_=tok_drams[2][:])
                nc.scalar.dma_start(out=tok_sb[3][:], in_=tok_drams[3][:])
                nc.gpsimd.dma_start(out=atwb_sb[:], in_=atwb[:])
                nc.scalar.dma_start(out=bias_sb[:], in_=bias[:])

        if wb_mode == "mm":
            # broadcast w down the partitions with K=1 ones-matmuls into PSUM
            # (512-col pieces: one PSUM bank per accumulation group). DVE
            # reads in1 straight from PSUM.
            ones_row = cpool.tile([1, P], BF16)
            nc.vector.memset(ones_row[:], 1.0)
            wb_ps = pspool.tile([P, 2 * H], F32)
            # pieces split at the w_src/w_tgt boundary (H=768), not at 512,
            # so w_src -- which gates chunk 0's mul-reduce -- is ready after
            # the first two (512+256 col) matmuls instead of two 512-col ones
            for j0, j1 in ((0, 512), (512, H), (H, 1024), (1024, 2 * H)):
                nc.tensor.matmul(wb_ps[:, j0:j1], ones_row[:, :P],
                                 wcat_sb[0:1, j0:j1], start=True, stop=True)
            wbs_ap = wb_ps[:, 0:H]
            wbt_ap = wb_ps[:, H:2 * H]
        else:
            wbs_ap = wbs_sb[:]
            wbt_ap = wbt_sb[:]

        if mm_mode == "mat":
            ones_pt = cpool.tile([P, P], BF16)
            nc.vector.memset(ones_pt[:], 1.0)

        psum_out = pspool.tile([S, T], F32)
        u_sb = cpool.tile([P, NCHUNK], F32)
        ub_bf = cpool.tile([P, NCHUNK], BF16)

        for c in range(NCHUNK):
            is_src = c < N_SRC_CHUNKS
            # u_c[t] = tok_c[t, :] . w  -- fused DVE multiply+reduce
            wb_col = wbs_ap if is_src else wbt_ap
            prod = ppool.tile([P, H], BF16, name=f"prod_{c}")
            if mr_op == "ttr":
                nc.vector.tensor_tensor_reduce(
                    out=prod[:], in0=tok_sb[c][:], in1=wb_col, scale=1.0,
                    scalar=0.0, op0=mybir.AluOpType.mult,
                    op1=mybir.AluOpType.add, accum_out=u_sb[:, c:c + 1])
            else:
                nc.vector.affine_mul_reduce(
                    out=prod[:], accum_out=u_sb[:, c:c + 1], in0=tok_sb[c][:],
                    in1=wb_col, scale=1.0, bias=0.0)

            atw_c = atwb_sb[:, c * P:(c + 1) * P]
            first = c == 0
            last = c == NCHUNK - 1
            if mm_mode == "mat":
                ub_mat = ppool.tile([P, P], BF16, name=f"ubm_{c}", tag="ubm")
                nc.vector.tensor_scalar_mul(ub_mat[:], ones_pt[:], u_sb[:, c:c + 1])
                rhs_b, lhs_b = ub_mat[:, :T], ub_mat[:, :S]
            else:
                nc.vector.tensor_copy(ub_bf[:, c:c + 1], u_sb[:, c:c + 1])
                rhs_b = ub_bf[:, c:c + 1].broadcast_to([P, T])
                lhs_b = ub_bf[:, c:c + 1].broadcast_to([P, S])
            if is_src:
                # psum[s, t] += sum_p atw[p, s] * u[p]   (same for all t)
                nc.tensor.matmul(psum_out[:], atw_c[:, :S], rhs_b,
                                 start=first, stop=last)
            else:
                nc.tensor.matmul(psum_out[:], lhs_b, atw_c[:, :T],
                                 start=first, stop=last)

        # final bias + PSUM->SBUF copy on the (idle) Activation engine so the
        # DVE chain's last op is the last membership matmul input
        out_sb = cpool.tile([S, T], BF16)
        nc.scalar.activation(out_sb[:], psum_out[:],
                             mybir.ActivationFunctionType.Identity,
                             bias=bias_sb[0:S, :], scale=1.0)
        # out rides the Act queue (idle by now; SP may still be draining toks)
        nc.scalar.dma_start(out=out[:], in_=out_sb[:])


# ---------------------------------------------------------------------------
# Device kernel, general fallback: tokens may map into either word block
# ---------------------------------------------------------------------------

def _build_general(nc, S, T):
    NW = S + T
    tok = nc.declare_dram_parameter("tok", [L, H], F32, isOutput=False)
    atw = nc.declare_dram_parameter("atw", [NCHUNK, P, NW], F32, isOutput=False)
    wcat = nc.declare_dram_parameter("wcat", [1, 2 * H + 1], F32, isOutput=False)
    out = nc.declare_dram_parameter("out", [S, T], F32, isOutput=True)

    with TileContext(nc) as tc:
        with (
            tc.tile_pool(name="const", bufs=1) as cpool,
            tc.tile_pool(name="toks", bufs=3) as tpool,
            tc.tile_pool(name="prods", bufs=2) as ppool,
            tc.tile_pool(name="atws", bufs=2) as apool,
            tc.tile_pool(name="psum", bufs=1, space="PSUM") as pspool,
        ):
            wcat_sb = cpool.tile([1, 2 * H + 1], F32)
            nc.scalar.dma_start(out=wcat_sb[:], in_=wcat[:])
            ones = cpool.tile([1, P], F32)
            nc.vector.memset(ones[:], 1.0)
            bias_row = cpool.tile([1, T], F32)
            nc.vector.tensor_scalar_mul(
                bias_row[:], ones[:, :T], wcat_sb[0:1, 2 * H:2 * H + 1])

            wb_src = pspool.tile([P, H], F32)
            wb_tgt = pspool.tile([P, H], F32)
            for wb, w0 in ((wb_src, 0), (wb_tgt, H)):
                for j0, j1 in ((0, 512), (512, H)):
                    nc.tensor.matmul(
                        wb[:, j0:j1], ones[:, :P], wcat_sb[0:1, w0 + j0:w0 + j1],
                        start=True, stop=True)

            psum_out = pspool.tile([S, T], F32)
            nc.tensor.matmul(psum_out[:], ones[:, :S], bias_row[:],
                             start=True, stop=False)

            u_src_sb = cpool.tile([P, NCHUNK], F32)
            u_tgt_sb = cpool.tile([P, NCHUNK], F32)
            for c in range(NCHUNK):
                tok_c = tpool.tile([P, H], F32, name=f"tok_{c}")
                nc.sync.dma_start(out=tok_c[:], in_=tok[c * P:(c + 1) * P, :])
                for kind, wb, usb in (("s", wb_src, u_src_sb), ("t", wb_tgt, u_tgt_sb)):
                    prod = ppool.tile([P, H], F32, name=f"prod_{kind}_{c}")
                    nc.vector.affine_mul_reduce(
                        out=prod[:], accum_out=usb[:, c:c + 1], in0=tok_c[:],
                        in1=wb[:], scale=1.0, bias=0.0)

                atw_c = apool.tile([P, NW], F32, name=f"atw_{c}")
                nc.sync.dma_start(out=atw_c[:], in_=atw[c])
                last = c == NCHUNK - 1
                nc.tensor.matmul(
                    psum_out[:], atw_c[:, :S], u_src_sb[:, c:c + 1].broadcast_to([P, T]),
                    start=False, stop=False)
                nc.tensor.matmul(
                    psum_out[:], u_tgt_sb[:, c:c + 1].broadcast_to([P, S]), atw_c[:, S:],
                    start=False, stop=last)

            out_sb = cpool.tile([S, T], F32)
            nc.vector.tensor_copy(out_sb[:], psum_out[:])
            nc.sync.dma_start(out=out[:], in_=out_sb[:])


# variant knobs (fixed at import for the graded path; bench overrides).
# Measured (same-process A/B, HW): bcast ~= mat (saves ~90ns/chunk of DVE);
# wb via TensorE ones-matmul broadcast with wcat leading the SP queue beats
# host-broadcast wb by ~0.6us (394KB less of the HBM-shared stream; earlier
# losses were from wcat landing late on the Act queue). The native
# TensorTensorReduce faults the HW (NRT_EXEC_UNIT_UNRECOVERABLE) despite
# passing CoreSim, so the custom affine_mul_reduce stays.
MM_MODE = "bcast"  # "mat" = DVE-manufactured u matrix, "bcast" = stride-0 AP
WB_MODE = "mm"     # "dma" = host-broadcast weights, "mm" = TensorE broadcast
MR_OP = "amr"      # "amr" = custom affine_mul_reduce, "ttr" = native TT-reduce


@functools.lru_cache(maxsize=8)
def _build(S, T, block_ok, mm_mode=None, wb_mode=None, mr_op=None):
    mm_mode = MM_MODE if mm_mode is None else mm_mode
    wb_mode = WB_MODE if wb_mode is None else wb_mode
    mr_op = MR_OP if mr_op is None else mr_op
    nc = bacc.Bacc("TRN2", debug=False, num_devices=N_CORES)
    if block_ok:
        prm = _declare_block_params(nc, S, T, wb_mode)
        with TileContext(nc) as tc:
            _emit_block_body(nc, tc, prm, S, T, mm_mode, wb_mode, mr_op)
    else:
        _build_general(nc, S, T)
    nc.compile()
    return nc


# ---------------------------------------------------------------------------
# Host wrapper
# ---------------------------------------------------------------------------

def _prep(inputs, wb_mode=None):
    wb_mode = WB_MODE if wb_mode is None else wb_mode
    tok_h = np.ascontiguousarray(np.asarray(inputs["tok_h"], dtype=np.float32))
    mask = np.asarray(inputs["attention_mask"])
    swid = np.asarray(inputs["source_word_ids"])
    twid = np.asarray(inputs["target_word_ids"])
    W = np.asarray(inputs["W"], dtype=np.float32)
    b = np.asarray(inputs["b"], dtype=np.float32)
    S = int(np.asarray(inputs["S"]))
    T = int(np.asarray(inputs["T"]))

    Bv, Lv, Hv = tok_h.shape
    assert (Bv, Lv, Hv) == (B, L, H), f"unexpected tok_h shape {tok_h.shape}"
    assert swid.shape == (B, L_SRC) and twid.shape == (B, L_TGT)
    assert S <= P and T <= P

    NW = S + T
    combined = np.concatenate([swid, twid], axis=1).astype(np.int64)
    seg, valid = _segments(combined, mask, NW)
    wgt = _seg_weights(seg, valid, NW)

    src_tok_seg = seg[:, :L_SRC][valid[:, :L_SRC]]
    tgt_tok_seg = seg[:, L_SRC:][valid[:, L_SRC:]]
    block_ok = bool(
        (src_tok_seg < S).all()
        and (tgt_tok_seg >= S).all() and (tgt_tok_seg < NW).all()
    )

    in_maps = []
    if block_ok:
        tok_bf = tok_h.astype(NPBF16)                       # [B, L, H]
        if wb_mode == "dma":
            wb_extra = {
                "wb_src": np.ascontiguousarray(
                    np.broadcast_to(W[:H, 0], (P, H))).astype(NPBF16),
                "wb_tgt": np.ascontiguousarray(
                    np.broadcast_to(W[H:2 * H, 0], (P, H))).astype(NPBF16),
            }
        else:
            wb_extra = {"wcat": W[:2 * H, 0].reshape(1, 2 * H).astype(NPBF16)}
        bias_col = np.full((P, 1), float(b.reshape(-1)[0]), dtype=np.float32)

        # atwb[b, p, c*P + col] = wgt for token c*P+p's in-block word col
        atwb = np.zeros((B, P, NCHUNK * P), dtype=np.float32)
        for bi in range(B):
            for c in range(NCHUNK):
                tsl = slice(c * P, (c + 1) * P)
                segc = seg[bi, tsl].astype(np.int64)
                col = segc if c < N_SRC_CHUNKS else segc - S
                ok = valid[bi, tsl] & (segc < NW) & (col >= 0) & (col < P)
                rows = np.arange(P)[ok]
                atwb[bi, rows, c * P + col[ok]] = wgt[bi, tsl][ok]
        atwb = atwb.astype(NPBF16)

        for i in range(N_CORES):
            bi = i % B
            tk = tok_bf[bi]
            in_maps.append({
                "tok0": np.ascontiguousarray(tk[0:P, :]),
                "tok1": np.ascontiguousarray(tk[P:2 * P, :]),
                "tok2": np.ascontiguousarray(tk[2 * P:3 * P, :]),
                "tok3": np.ascontiguousarray(tk[3 * P:4 * P, :]),
                "atwb": atwb[bi],
                "bias": bias_col,
                **wb_extra,
            })
    else:
        wcat = np.zeros((1, 2 * H + 1), dtype=np.float32)
        wcat[0, :H] = W[:H, 0]
        wcat[0, H:2 * H] = W[H:2 * H, 0]
        wcat[0, 2 * H] = b.reshape(-1)[0]
        atw = np.zeros((B, NCHUNK, P, NW), dtype=np.float32)
        for bi in range(B):
            for t in range(L):
                s = seg[bi, t]
                if s >= NW or not valid[bi, t]:
                    continue
                atw[bi, t // P, t % P, s] = wgt[bi, t]
        for i in range(N_CORES):
            bi = i % B
            in_maps.append({"tok": tok_h[bi], "atw": atw[bi], "wcat": wcat})
    return S, T, block_ok, in_maps


def kernel(**inputs):
    S, T, block_ok, in_maps = _prep(inputs)
    nc = _build(S, T, block_ok)
    res = run_bass_kernel_spmd(nc, in_maps, core_ids=list(range(N_CORES)))
    return np.stack(
        [np.asarray(res.results[i]["out"]).astype(np.float32) for i in range(B)],
        axis=0)
